# revision 35
# baseline (speedup 1.0000x reference)
"""Trainium2 Bass kernel for nn_HadamardExpansionV2 (topk_masking).

Reference computation:
  mask  = hard gumbel-softmax over c1=256, for 2*ce rows  -> numerically an
          exact one-hot matrix scaled by w=(1-s)+s (w==1.0 in fp32 for all rows)
  x_i   = einsum('ec,bcl->bel', mask[0], x)   == gather of channels i0[e]
  x_j   = einsum('ec,bcl->bel', mask[1], x)   == gather of channels i1[e]
  xe    = x_i * x_j                            [B, ce, H, W]
  out   = BatchNorm2d(train mode, batch stats over (B,H,W)) * gamma + beta

Strategy (8 NeuronCores, no collectives):
  - Shard the ce=512 expanded channels: core k owns e in [64k, 64k+64).
  - Host computes argmax indices from (logits+gumbel)/tau and pre-gathers the
    channel pairs into a per-core dense f16 tensor xsel [128, B*L]:
    row s<64 -> x[:, i0[e0+s], :], row s>=64 -> x[:, i1[e0+s-64], :].
    BatchNorm stats for a given e are then fully local to one core.
  - Device (identical program on all 8 cores), per group g of 8 e's
    (partition = (e_sub, b), free = l):
      DVE  tensor_tensor: prod = xi*xj (f16, 2x fast mode)
      PE   7 accumulated matmuls rmat16^T x prod -> S_ps[8, 448] (sum over
           the 16 b-partitions of each e and over column chunks)
      DVE  tensor_reduce S_ps -> per-e S (tiny)
      ACT  Square -> PSUM scratch w/ accum_out: per-partition sumsq SS
    Stats finalization batched NB=2 groups at a time:
      PE   matmul with block-one-hot R: per-e SS
      ~10 small DVE ops -> A = w*gamma*rstd, B = beta - A*mean
      PE   matmul with R^T broadcasts A,B -> ab_vec [128, 2*NB]
      DVE  tensor_scalar full-width: out = prod*A + B (f16), lagged 2 groups
      DMA  (gpsimd/SWDGE) one out tile per group -> out[e, b, l] f16
  - DMA totals ~19.3MB/core (12.85 in + 6.4 out) -> ~50us at the ~390GB/s
    16-engine DMA bus; DVE ~35us, ACT ~31us, PE ~20us busy underneath.
  - Mask weight w is folded via host-precomputed coef columns (w/N, w^2/N,
    gamma*w, beta), so the general (non-exactly-one-hot) path stays exact.

The bass program depends only on shapes -> compiled once and cached.
"""

import os
import sys
from contextlib import ExitStack

import numpy as np

sys.path.insert(0, "/opt/trn_rl_repo")

import concourse.bass as bass  # noqa: E402
import concourse.tile as tile  # noqa: E402
import concourse.mybir as mybir  # noqa: E402
from concourse import bacc  # noqa: E402
from concourse.bass_utils import run_bass_kernel_spmd  # noqa: E402

# Problem shapes (hardcoded per contract)
B, C1, H, W = 16, 256, 56, 56
L = H * W                      # 3136
CE = 512
NCORES = 8
EPC = CE // NCORES             # 64 e-channels per core
NG = 8                         # groups per core
EG = EPC // NG                 # 8 e-channels per group
NB = 2                         # groups per stats batch
N = B * L                      # 50176 elements per channel for BN stats
BN_EPS = 1e-5

F32 = mybir.dt.float32
F16 = mybir.dt.float16

# gather dtype: "f32" (exact) or "f16" (~3e-4 rel err, half the DMA).
# The rel-err gate is 2e-2, so f16 is the default.
GATHER_DTYPE = os.environ.get("KERNEL_GATHER_DTYPE", "f16")
# output dtype: f16 halves the output DMA; host converts back to f32.
OUT_DTYPE = os.environ.get("KERNEL_OUT_DTYPE", "f16")

# packed consts layout (single [128, CCOLS] f32 input):
#   cols 0:8     rmat   [128, EG]   block-one-hot for per-e stats matmul
#   cols 8:40    coef   [EG, 4*NG]  (partitions 0:EG): w/N, w^2/N, g*w, beta
#   cols 40:168  rtmat  [EG, 128]   (partitions 0:EG)
#   col  168     eps    [EG, 1]
CCOLS = 169

_PROGRAMS = {}  # (gdt, odt) -> compiled program
LAST_RESULT = None  # BassKernelResults of the most recent run (for profiling)


def _build_program(gdt_name, odt_name):
    """Build + compile the (shape-only) bass program shared by all cores."""
    gdt = F16 if gdt_name == "f16" else F32
    odt = F16 if odt_name == "f16" else F32
    nc = bacc.Bacc("TRN2", target_bir_lowering=False, debug=False,
                   num_devices=NCORES)

    xsel_d = nc.dram_tensor("xsel", [128, N], gdt, kind="ExternalInput").ap()
    consts_d = nc.dram_tensor("consts", [128, CCOLS], F32,
                              kind="ExternalInput").ap()
    # e-major output: each group's [128, L] tile lands as one contiguous
    # block; host transposes back to [B, EPC, L].
    out_d = nc.dram_tensor("out", [EPC, B, L], odt, kind="ExternalOutput").ap()

    # views: xsel[(m g e), (b l)] -> [m, g, (e b), l]
    xsel_r = xsel_d.rearrange("(m g e) (b l) -> m g (e b) l", m=2, g=NG, b=B)
    # out[(g e), b, l] -> [g, (e b), l]
    out_r = out_d.rearrange("(g e) b l -> g (e b) l", g=NG)

    LH = L // 2
    M = mybir.AluOpType.mult
    SUB = mybir.AluOpType.subtract

    with tile.TileContext(nc) as tc, ExitStack() as ctx:
        const_pool = ctx.enter_context(tc.tile_pool(name="consts", bufs=1))
        xio_pool = ctx.enter_context(tc.tile_pool(name="xio", bufs=4))
        prod_pool = ctx.enter_context(tc.tile_pool(name="prod", bufs=NB + 3))
        out_pool = ctx.enter_context(tc.tile_pool(name="outs", bufs=3))
        stats_pool = ctx.enter_context(tc.tile_pool(name="stats", bufs=2))
        small_pool = ctx.enter_context(tc.tile_pool(name="smalls", bufs=2))
        vec_pool = ctx.enter_context(tc.tile_pool(name="vecs", bufs=2))
        psum_sq_pool = ctx.enter_context(
            tc.tile_pool(name="psum_sq", bufs=1, space="PSUM"))
        psum_pool = ctx.enter_context(
            tc.tile_pool(name="psum", bufs=1, space="PSUM"))

        # packed consts, one DMA; rmat16 (f16 copy for PE-on-prod matmuls)
        c_t = const_pool.tile([128, CCOLS], F32)
        nc.sync.dma_start(c_t[:], consts_d[:])
        r_sb = c_t[:, 0:EG]                      # [128, EG] f32
        rt_sb = c_t[0:EG, 40:168]                # [EG, 128]
        eps_t = c_t[0:EG, 168:169]               # [EG, 1]

        def coef_cols(row, g0, n):
            c0 = 8 + row * NG + g0
            return c_t[0:EG, c0:c0 + n]

        prods = {}     # g -> prod tile
        ab_vecs = {}   # batch q -> ab_vec [128, 2*NB] (A cols then B cols)
        st_tiles = {}  # batch q -> S/SS accum slots [128, 4*NB]

        def do_norm(g, act_cols=0):
            """Normalize group g; the last act_cols columns go to ACT (used
            in the epilogue where ACT is otherwise idle, and for a small
            mid-pipe slice to keep DVE under the input cadence)."""
            q, j = divmod(g, NB)
            ab_vec = ab_vecs[q]
            av = ab_vec[:, j:j + 1]
            bv = ab_vec[:, NB + j:NB + j + 1]
            out_t = out_pool.tile([128, L], odt, tag="outt")
            split = L - act_cols
            nc.vector.tensor_scalar(out=out_t[:, 0:split],
                                    in0=prods[g][:, 0:split],
                                    scalar1=av, scalar2=bv,
                                    op0=M, op1=mybir.AluOpType.add)
            if act_cols:
                nc.scalar.activation(
                    out=out_t[:, split:L], in_=prods[g][:, split:L],
                    func=mybir.ActivationFunctionType.Identity,
                    scale=av, bias=bv)
            # halves: two smaller SWDGE transfers drain much better than one
            # full-tile transfer on the gpsimd DMA queues
            nc.gpsimd.dma_start(out_r[g][:, 0:LH], out_t[:, 0:LH])
            nc.gpsimd.dma_start(out_r[g][:, LH:L], out_t[:, LH:L])

        def finalize(q):
            """Stats finalize for groups NB*q..NB*q+NB-1 -> ab_vecs[q]."""
            g0 = q * NB
            agg_ps = psum_pool.tile([EG, 4 * NB], F32, tag="agg")
            nc.tensor.matmul(agg_ps[:], r_sb[:], st_tiles[q][:],
                             start=True, stop=True)

            sm = small_pool.tile([EG, 9 * NB], F32, tag="sm")
            s_sum = sm[:, 0 * NB:1 * NB]
            ss_sum = sm[:, 1 * NB:2 * NB]
            mw = sm[:, 2 * NB:3 * NB]
            msn = sm[:, 3 * NB:4 * NB]
            mwsq = sm[:, 4 * NB:5 * NB]
            nvar = sm[:, 5 * NB:6 * NB]
            sd = sm[:, 6 * NB:7 * NB]
            rstd = sm[:, 7 * NB:8 * NB]
            mean = sm[:, 8 * NB:9 * NB]
            t = mwsq                             # reuse slot as scratch
            ab = small_pool.tile([EG, 2 * NB], F32, tag="ab")
            wn = coef_cols(0, g0, NB)            # w/N
            wsqn = coef_cols(1, g0, NB)          # w^2/N
            gw = coef_cols(2, g0, NB)            # gamma*w
            bet = coef_cols(3, g0, NB)           # beta

            # S = slots 0+1, SS = slots 2+3 per group (pairwise strided add)
            agg = small_pool.tile([EG, 4 * NB], F32, tag="agg_sb")
            nc.vector.tensor_copy(agg[:], agg_ps[:])
            ag = agg[:]
            nc.vector.tensor_add(s_sum, ag[:, 0:4 * NB:4], ag[:, 1:4 * NB:4])
            nc.vector.tensor_add(ss_sum, ag[:, 2:4 * NB:4],
                                 ag[:, 3:4 * NB:4])
            # mw = w*mean = S*(w/N) ; msn = SS*(w^2/N)
            nc.vector.tensor_tensor(out=mw, in0=s_sum, in1=wn, op=M)
            nc.vector.tensor_tensor(out=msn, in0=ss_sum, in1=wsqn, op=M)
            # nvar = mw^2 - msn = -var'
            nc.vector.tensor_tensor(out=mwsq, in0=mw, in1=mw, op=M)
            nc.vector.tensor_tensor(out=nvar, in0=mwsq, in1=msn, op=SUB)
            # sd = sqrt(var' + eps) = sqrt(-1*nvar + eps)
            nc.scalar.activation(out=sd, in_=nvar,
                                 func=mybir.ActivationFunctionType.Sqrt,
                                 scale=-1.0, bias=eps_t)
            nc.vector.reciprocal(rstd, sd)
            # A = rstd*(gamma*w) ; B = beta - A*mean  (mean = S/N)
            nc.vector.tensor_tensor(out=ab[:, 0:NB], in0=rstd, in1=gw, op=M)
            nc.vector.tensor_scalar(out=mean, in0=s_sum,
                                    scalar1=float(np.float32(1.0 / N)),
                                    scalar2=None, op0=M)
            nc.vector.tensor_tensor(out=t, in0=ab[:, 0:NB], in1=mean, op=M)
            nc.vector.tensor_tensor(out=ab[:, NB:2 * NB], in0=bet, in1=t,
                                    op=SUB)

            # broadcast A,B to per-partition vectors [128, 2*NB]
            bc_ps = psum_pool.tile([128, 2 * NB], F32, tag="bc")
            nc.tensor.matmul(bc_ps[:], rt_sb[:], ab[:],
                             start=True, stop=True)
            ab_vec = vec_pool.tile([128, 2 * NB], F32, tag="abv")
            nc.vector.tensor_copy(ab_vec[:], bc_ps[:])
            ab_vecs[q] = ab_vec

        for g in range(NG):
            q, j = divmod(g, NB)
            # ---- gather inputs for this group (ring depth 4 prefetches) ----
            xi_t = xio_pool.tile([128, L], gdt, tag="xi")
            nc.sync.dma_start(xi_t[:], xsel_r[0, g])
            xj_t = xio_pool.tile([128, L], gdt, tag="xj")
            nc.sync.dma_start(xj_t[:], xsel_r[1, g])

            if j == 0:
                st_tiles[q] = stats_pool.tile([128, 4 * NB], F32, tag="st",
                                              name=f"st{q}")
            st = st_tiles[q]
            prod_t = prod_pool.tile([128, L], gdt, tag="prod")
            prods[g] = prod_t

            # ---- pipelined work from earlier groups, issued while this
            # group's inputs are still streaming in:
            #   finalize(q') at group 2q'+3: all its inputs (S/SS accums of
            #   group 2q'+1) completed during group 2q'+2, so neither DVE
            #   nor ACT stalls on the cross-engine chain.
            #   norms lag 3 groups behind.
            if g >= 3 and g % NB == 1:
                finalize((g - 3) // NB)

            # ---- prod = xi*xj with fused per-partition S accum (halves:
            # lets the ACT Square of h0 overlap the stt of h1) ----
            for h in range(2):
                cs = slice(h * LH, (h + 1) * LH)
                nc.vector.scalar_tensor_tensor(
                    out=prod_t[:, cs],
                    in0=xi_t[:, cs],
                    scalar=1.0,
                    in1=xj_t[:, cs],
                    op0=M, op1=M,
                    accum_out=st[:, 4 * j + h:4 * j + h + 1],
                )

            if g >= NB + 1:
                do_norm(g - NB - 1, act_cols=512)

            # ---- SS accum via ACT Square -> PSUM scratch ----
            for h in range(2):
                cs = slice(h * LH, (h + 1) * LH)
                sq_ps = psum_sq_pool.tile([128, LH], F32, tag="sq")
                nc.scalar.activation(
                    out=sq_ps[:],
                    in_=prod_t[:, cs],
                    func=mybir.ActivationFunctionType.Square,
                    accum_out=st[:, 4 * j + 2 + h:4 * j + 3 + h],
                )

        # epilogue: drain the pipeline (norm lag is NB+1 = 3); ACT is idle
        # here, so it takes a bigger share of the last normalizes
        finalize(NG // NB - 1)
        for g in range(NG - NB - 1, NG):
            do_norm(g, act_cols=1024)

    nc.compile()
    return nc


def _get_program(gdt_name=None, odt_name=None):
    gdt_name = gdt_name or GATHER_DTYPE
    odt_name = odt_name or OUT_DTYPE
    key = (gdt_name, odt_name)
    if key not in _PROGRAMS:
        _PROGRAMS[key] = _build_program(gdt_name, odt_name)
    return _PROGRAMS[key]


def _host_prep(x, logits, gumbel, tau, gamma, beta):
    """Compute mask indices/weights and build per-core inputs."""
    x = np.asarray(x, dtype=np.float32)
    logits = np.asarray(logits, dtype=np.float32)
    gumbel = np.asarray(gumbel, dtype=np.float32)
    tau_f = np.float32(np.asarray(tau))
    gamma = np.asarray(gamma, dtype=np.float32)
    beta = np.asarray(beta, dtype=np.float32)

    # replicate reference softmax/argmax in fp32 (argmax of z == argmax of
    # softmax(z); verified min top-2 gap 3.4e-4 for these inputs)
    z = (logits + gumbel) / tau_f                     # [2, CE, C1] fp32
    idx = z.argmax(axis=-1)                           # [2, CE]
    zm = z.max(axis=-1, keepdims=True)
    ez = np.exp(z - zm, dtype=np.float32)
    soft = ez / ez.sum(axis=-1, keepdims=True, dtype=np.float32)
    s_hot = np.take_along_axis(soft, idx[..., None], axis=-1)[..., 0]
    w = (np.float32(1.0) - s_hot) + s_hot             # [2, CE] (== 1.0 here)
    weff = (w[0] * w[1]).astype(np.float32)           # [CE]

    inv_n = np.float32(1.0) / np.float32(N)

    # channel-major copy of x for fast row gathers: [C1, B*L]
    xt = np.ascontiguousarray(
        x.reshape(B, C1, L).transpose(1, 0, 2)).reshape(C1, N)
    if GATHER_DTYPE == "f16":
        xt = xt.astype(np.float16)

    # R / R^T block one-hot (partition p belongs to e_sub = p//B)
    rmat = np.zeros((128, EG), dtype=np.float32)
    for es in range(EG):
        rmat[es * B:(es + 1) * B, es] = 1.0
    rtmat = np.ascontiguousarray(rmat.T)

    in_maps = []
    for k in range(NCORES):
        e0 = k * EPC
        rows = np.concatenate([idx[0, e0:e0 + EPC], idx[1, e0:e0 + EPC]])
        xsel = np.ascontiguousarray(xt[rows])         # [128, N]

        coef = np.zeros((EG, 4 * NG), dtype=np.float32)
        for g in range(NG):
            el = e0 + g * EG + np.arange(EG)          # global e for (g, e_sub)
            we = weff[el]
            coef[:, 0 * NG + g] = we * inv_n                  # w/N
            coef[:, 1 * NG + g] = we * we * inv_n             # w^2/N
            coef[:, 2 * NG + g] = gamma[el] * we              # gamma*w
            coef[:, 3 * NG + g] = beta[el]

        consts = np.zeros((128, CCOLS), dtype=np.float32)
        consts[:, 0:EG] = rmat
        consts[0:EG, 8:40] = coef
        consts[0:EG, 40:168] = rtmat
        consts[0:EG, 168] = BN_EPS

        in_maps.append({
            "xsel": xsel,
            "consts": consts,
        })
    return in_maps


def _install_ntff_shim():
    """The agent image's antenv lacks axon_hooks; recreate it so
    run_bass_kernel_spmd(trace=True) can capture NTFF profiles."""
    import types
    if "antenv.axon_hooks" in sys.modules:
        return
    mod = types.ModuleType("antenv.axon_hooks")
    _hook = [None]
    mod.set_axon_ntff_profile_hook = lambda h: _hook.__setitem__(0, h)
    mod.get_axon_ntff_profile_hook = lambda: _hook[0]
    sys.modules["antenv.axon_hooks"] = mod
    import antenv
    antenv.axon_hooks = mod
    from trn_agent_boot.trn_boot import _ntff_profile_via_ctypes
    mod.set_axon_ntff_profile_hook(
        _ntff_profile_via_ctypes("/opt/axon/libaxon_pjrt.so"))


def kernel(x, logits, gumbel, tau, gamma, beta):
    global LAST_RESULT
    nc = _get_program()
    in_maps = _host_prep(x, logits, gumbel, tau, gamma, beta)

    trace = bool(int(os.environ.get("KERNEL_PROFILE", "0")))
    if trace:
        try:
            _install_ntff_shim()
        except Exception:
            trace = False
    try:
        res = run_bass_kernel_spmd(nc, in_maps, list(range(NCORES)),
                                   trace=trace)
    except Exception:
        if not trace:
            raise
        res = run_bass_kernel_spmd(nc, in_maps, list(range(NCORES)),
                                   trace=False)
    LAST_RESULT = res

    out = np.empty((B, CE, L), dtype=np.float32)
    for k in range(NCORES):
        ok = res.results[k]["out"]
        if ok.dtype != np.float32:
            ok = ok.astype(np.float32)
        out[:, k * EPC:(k + 1) * EPC, :] = ok.transpose(1, 0, 2)
    return out.reshape(B, CE, H, W)


# revision 38
# speedup vs baseline: 1.0039x; 1.0039x over previous
"""Trainium2 Bass kernel for nn_HadamardExpansionV2 (topk_masking).

Reference computation:
  mask  = hard gumbel-softmax over c1=256, for 2*ce rows  -> numerically an
          exact one-hot matrix scaled by w=(1-s)+s (w==1.0 in fp32 for all rows)
  x_i   = einsum('ec,bcl->bel', mask[0], x)   == gather of channels i0[e]
  x_j   = einsum('ec,bcl->bel', mask[1], x)   == gather of channels i1[e]
  xe    = x_i * x_j                            [B, ce, H, W]
  out   = BatchNorm2d(train mode, batch stats over (B,H,W)) * gamma + beta

Strategy (8 NeuronCores, no collectives):
  - Shard the ce=512 expanded channels: core k owns e in [64k, 64k+64).
  - Host computes argmax indices from (logits+gumbel)/tau and pre-gathers the
    channel pairs into a per-core dense f16 tensor xsel [128, B*L]:
    row s<64 -> x[:, i0[e0+s], :], row s>=64 -> x[:, i1[e0+s-64], :].
    BatchNorm stats for a given e are then fully local to one core.
  - Device (identical program on all 8 cores), per group g of 8 e's
    (partition = (e_sub, b), free = l):
      DVE  scalar_tensor_tensor halves: prod = xi*xj (f16) with fused
           per-partition accum S (2-tensor DVE ops run 1x regardless of
           the accumulator, so S rides along for free)
      ACT  Square -> PSUM scratch w/ accum_out: per-partition sumsq SS
    Stats finalization batched NB=2 groups at a time, issued at group
    2q+3 so every input is already complete (no cross-engine stalls):
      PE   matmul with block-one-hot R: per-e S,SS sums over partitions
      ~12 small DVE ops -> A = rstd*gamma*w, B = beta - A*mean
      PE   matmul with R^T broadcasts A,B -> ab_vec [128, 2*NB]
      DVE  tensor_scalar full-width: out = prod*A + B (f16), lagged 3
           groups behind the input stream (software pipeline)
      DMA  (gpsimd/SWDGE, idle engine) out tile halves -> out[e, b, l]
  - DMA totals ~19.3MB/core (12.85 in + 6.4 out) -> ~50us at the ~390GB/s
    16-engine DMA bus; DVE ~41us, ACT ~31us busy underneath. Empirically
    the DMA engines are a shared bus for HBM AND SBUF<->SBUF traffic, so
    on-chip dedup/duplication cannot beat the host pre-gather; and SBUF
    port bandwidth is the second-order constraint (extra PE passes over
    prod slow DVE ops by ~20%, which is why S is fused into the stt).
  - Mask weight w is folded via host-precomputed coef columns (w/N, w^2/N,
    gamma*w, beta), so the general (non-exactly-one-hot) path stays exact.

The bass program depends only on shapes -> compiled once and cached.
"""

import os
import sys
from contextlib import ExitStack

import numpy as np

sys.path.insert(0, "/opt/trn_rl_repo")

import concourse.bass as bass  # noqa: E402
import concourse.tile as tile  # noqa: E402
import concourse.mybir as mybir  # noqa: E402
from concourse import bacc  # noqa: E402
from concourse.bass_utils import run_bass_kernel_spmd  # noqa: E402

# Problem shapes (hardcoded per contract)
B, C1, H, W = 16, 256, 56, 56
L = H * W                      # 3136
CE = 512
NCORES = 8
EPC = CE // NCORES             # 64 e-channels per core
NG = 8                         # groups per core
EG = EPC // NG                 # 8 e-channels per group
NB = 2                         # groups per stats batch
N = B * L                      # 50176 elements per channel for BN stats
BN_EPS = 1e-5

F32 = mybir.dt.float32
F16 = mybir.dt.float16

# gather dtype: "f32" (exact) or "f16" (~3e-4 rel err, half the DMA).
# The rel-err gate is 2e-2, so f16 is the default.
GATHER_DTYPE = os.environ.get("KERNEL_GATHER_DTYPE", "f16")
# output dtype: f16 halves the output DMA; host converts back to f32.
OUT_DTYPE = os.environ.get("KERNEL_OUT_DTYPE", "f16")

# packed consts layout (single [128, CCOLS] f32 input):
#   cols 0:8     rmat   [128, EG]   block-one-hot for per-e stats matmul
#   cols 8:40    coef   [EG, 4*NG]  (partitions 0:EG): w/N, w^2/N, g*w, beta
#   cols 40:168  rtmat  [EG, 128]   (partitions 0:EG)
#   col  168     eps    [EG, 1]
CCOLS = 169

_PROGRAMS = {}  # (gdt, odt) -> compiled program
LAST_RESULT = None  # BassKernelResults of the most recent run (for profiling)


def _build_program(gdt_name, odt_name):
    """Build + compile the (shape-only) bass program shared by all cores."""
    gdt = F16 if gdt_name == "f16" else F32
    odt = F16 if odt_name == "f16" else F32
    nc = bacc.Bacc("TRN2", target_bir_lowering=False, debug=False,
                   num_devices=NCORES)

    xsel_d = nc.dram_tensor("xsel", [128, N], gdt, kind="ExternalInput").ap()
    consts_d = nc.dram_tensor("consts", [128, CCOLS], F32,
                              kind="ExternalInput").ap()
    # e-major output: each group's [128, L] tile lands as one contiguous
    # block; host transposes back to [B, EPC, L].
    out_d = nc.dram_tensor("out", [EPC, B, L], odt, kind="ExternalOutput").ap()

    # views: xsel[(m g e), (b l)] -> [m, g, (e b), l]
    xsel_r = xsel_d.rearrange("(m g e) (b l) -> m g (e b) l", m=2, g=NG, b=B)
    # out[(g e), b, l] -> [g, (e b), l]
    out_r = out_d.rearrange("(g e) b l -> g (e b) l", g=NG)

    LH = L // 2
    M = mybir.AluOpType.mult
    SUB = mybir.AluOpType.subtract

    with tile.TileContext(nc) as tc, ExitStack() as ctx:
        const_pool = ctx.enter_context(tc.tile_pool(name="consts", bufs=1))
        xio_pool = ctx.enter_context(tc.tile_pool(name="xio", bufs=4))
        prod_pool = ctx.enter_context(tc.tile_pool(name="prod", bufs=NB + 3))
        out_pool = ctx.enter_context(tc.tile_pool(name="outs", bufs=3))
        stats_pool = ctx.enter_context(tc.tile_pool(name="stats", bufs=2))
        small_pool = ctx.enter_context(tc.tile_pool(name="smalls", bufs=2))
        vec_pool = ctx.enter_context(tc.tile_pool(name="vecs", bufs=2))
        psum_sq_pool = ctx.enter_context(
            tc.tile_pool(name="psum_sq", bufs=1, space="PSUM"))
        psum_pool = ctx.enter_context(
            tc.tile_pool(name="psum", bufs=1, space="PSUM"))

        # packed consts, one DMA; rmat16 (f16 copy for PE-on-prod matmuls)
        c_t = const_pool.tile([128, CCOLS], F32)
        nc.sync.dma_start(c_t[:], consts_d[:])
        r_sb = c_t[:, 0:EG]                      # [128, EG] f32
        rt_sb = c_t[0:EG, 40:168]                # [EG, 128]
        eps_t = c_t[0:EG, 168:169]               # [EG, 1]

        def coef_cols(row, g0, n):
            c0 = 8 + row * NG + g0
            return c_t[0:EG, c0:c0 + n]

        prods = {}     # g -> prod tile
        ab_vecs = {}   # batch q -> ab_vec [128, 2*NB] (A cols then B cols)
        st_tiles = {}  # batch q -> S/SS accum slots [128, 4*NB]

        def do_norm(g, act_cols=0):
            """Normalize group g; the last act_cols columns go to ACT (used
            in the epilogue where ACT is otherwise idle, and for a small
            mid-pipe slice to keep DVE under the input cadence)."""
            q, j = divmod(g, NB)
            ab_vec = ab_vecs[q]
            av = ab_vec[:, j:j + 1]
            bv = ab_vec[:, NB + j:NB + j + 1]
            out_t = out_pool.tile([128, L], odt, tag="outt")
            split = L - act_cols
            nc.vector.tensor_scalar(out=out_t[:, 0:split],
                                    in0=prods[g][:, 0:split],
                                    scalar1=av, scalar2=bv,
                                    op0=M, op1=mybir.AluOpType.add)
            if act_cols:
                nc.scalar.activation(
                    out=out_t[:, split:L], in_=prods[g][:, split:L],
                    func=mybir.ActivationFunctionType.Identity,
                    scale=av, bias=bv)
            # halves: two smaller SWDGE transfers drain much better than one
            # full-tile transfer on the gpsimd DMA queues
            nc.gpsimd.dma_start(out_r[g][:, 0:LH], out_t[:, 0:LH])
            nc.gpsimd.dma_start(out_r[g][:, LH:L], out_t[:, LH:L])

        def finalize(q):
            """Stats finalize for groups NB*q..NB*q+NB-1 -> ab_vecs[q]."""
            g0 = q * NB
            agg_ps = psum_pool.tile([EG, 4 * NB], F32, tag="agg")
            nc.tensor.matmul(agg_ps[:], r_sb[:], st_tiles[q][:],
                             start=True, stop=True)

            sm = small_pool.tile([EG, 9 * NB], F32, tag="sm")
            s_sum = sm[:, 0 * NB:1 * NB]
            ss_sum = sm[:, 1 * NB:2 * NB]
            mw = sm[:, 2 * NB:3 * NB]
            msn = sm[:, 3 * NB:4 * NB]
            mwsq = sm[:, 4 * NB:5 * NB]
            nvar = sm[:, 5 * NB:6 * NB]
            sd = sm[:, 6 * NB:7 * NB]
            rstd = sm[:, 7 * NB:8 * NB]
            mean = sm[:, 8 * NB:9 * NB]
            t = mwsq                             # reuse slot as scratch
            ab = small_pool.tile([EG, 2 * NB], F32, tag="ab")
            wn = coef_cols(0, g0, NB)            # w/N
            wsqn = coef_cols(1, g0, NB)          # w^2/N
            gw = coef_cols(2, g0, NB)            # gamma*w
            bet = coef_cols(3, g0, NB)           # beta

            # S = slots 0+1, SS = slots 2+3 per group (pairwise strided add)
            agg = small_pool.tile([EG, 4 * NB], F32, tag="agg_sb")
            nc.vector.tensor_copy(agg[:], agg_ps[:])
            ag = agg[:]
            nc.vector.tensor_add(s_sum, ag[:, 0:4 * NB:4], ag[:, 1:4 * NB:4])
            nc.vector.tensor_add(ss_sum, ag[:, 2:4 * NB:4],
                                 ag[:, 3:4 * NB:4])
            # mw = w*mean = S*(w/N) ; msn = SS*(w^2/N)
            nc.vector.tensor_tensor(out=mw, in0=s_sum, in1=wn, op=M)
            nc.vector.tensor_tensor(out=msn, in0=ss_sum, in1=wsqn, op=M)
            # nvar = mw^2 - msn = -var'
            nc.vector.tensor_tensor(out=mwsq, in0=mw, in1=mw, op=M)
            nc.vector.tensor_tensor(out=nvar, in0=mwsq, in1=msn, op=SUB)
            # sd = sqrt(var' + eps) = sqrt(-1*nvar + eps)
            nc.scalar.activation(out=sd, in_=nvar,
                                 func=mybir.ActivationFunctionType.Sqrt,
                                 scale=-1.0, bias=eps_t)
            nc.vector.reciprocal(rstd, sd)
            # A = rstd*(gamma*w) ; B = beta - A*mean  (mean = S/N)
            nc.vector.tensor_tensor(out=ab[:, 0:NB], in0=rstd, in1=gw, op=M)
            nc.vector.tensor_scalar(out=mean, in0=s_sum,
                                    scalar1=float(np.float32(1.0 / N)),
                                    scalar2=None, op0=M)
            nc.vector.tensor_tensor(out=t, in0=ab[:, 0:NB], in1=mean, op=M)
            nc.vector.tensor_tensor(out=ab[:, NB:2 * NB], in0=bet, in1=t,
                                    op=SUB)

            # broadcast A,B to per-partition vectors [128, 2*NB]
            bc_ps = psum_pool.tile([128, 2 * NB], F32, tag="bc")
            nc.tensor.matmul(bc_ps[:], rt_sb[:], ab[:],
                             start=True, stop=True)
            ab_vec = vec_pool.tile([128, 2 * NB], F32, tag="abv")
            nc.vector.tensor_copy(ab_vec[:], bc_ps[:])
            ab_vecs[q] = ab_vec

        for g in range(NG):
            q, j = divmod(g, NB)
            # ---- gather inputs for this group (ring depth 4 prefetches) ----
            xi_t = xio_pool.tile([128, L], gdt, tag="xi")
            nc.sync.dma_start(xi_t[:], xsel_r[0, g])
            xj_t = xio_pool.tile([128, L], gdt, tag="xj")
            nc.sync.dma_start(xj_t[:], xsel_r[1, g])

            if j == 0:
                st_tiles[q] = stats_pool.tile([128, 4 * NB], F32, tag="st",
                                              name=f"st{q}")
            st = st_tiles[q]
            prod_t = prod_pool.tile([128, L], gdt, tag="prod")
            prods[g] = prod_t

            # ---- pipelined work from earlier groups, issued while this
            # group's inputs are still streaming in:
            #   finalize(q') at group 2q'+3: all its inputs (S/SS accums of
            #   group 2q'+1) completed during group 2q'+2, so neither DVE
            #   nor ACT stalls on the cross-engine chain.
            #   norms lag 3 groups behind.
            if g >= 3 and g % NB == 1:
                finalize((g - 3) // NB)

            # ---- prod = xi*xj with fused per-partition S accum (halves:
            # lets the ACT Square of h0 overlap the stt of h1) ----
            for h in range(2):
                cs = slice(h * LH, (h + 1) * LH)
                nc.vector.scalar_tensor_tensor(
                    out=prod_t[:, cs],
                    in0=xi_t[:, cs],
                    scalar=1.0,
                    in1=xj_t[:, cs],
                    op0=M, op1=M,
                    accum_out=st[:, 4 * j + h:4 * j + h + 1],
                )

            if g >= NB + 1:
                do_norm(g - NB - 1)

            # ---- SS accum via ACT Square -> PSUM scratch ----
            for h in range(2):
                cs = slice(h * LH, (h + 1) * LH)
                sq_ps = psum_sq_pool.tile([128, LH], F32, tag="sq")
                nc.scalar.activation(
                    out=sq_ps[:],
                    in_=prod_t[:, cs],
                    func=mybir.ActivationFunctionType.Square,
                    accum_out=st[:, 4 * j + 2 + h:4 * j + 3 + h],
                )

        # epilogue: drain the pipeline (norm lag is NB+1 = 3)
        finalize(NG // NB - 1)
        for g in range(NG - NB - 1, NG):
            do_norm(g)

    nc.compile()
    return nc


def _get_program(gdt_name=None, odt_name=None):
    gdt_name = gdt_name or GATHER_DTYPE
    odt_name = odt_name or OUT_DTYPE
    key = (gdt_name, odt_name)
    if key not in _PROGRAMS:
        _PROGRAMS[key] = _build_program(gdt_name, odt_name)
    return _PROGRAMS[key]


def _host_prep(x, logits, gumbel, tau, gamma, beta):
    """Compute mask indices/weights and build per-core inputs."""
    x = np.asarray(x, dtype=np.float32)
    logits = np.asarray(logits, dtype=np.float32)
    gumbel = np.asarray(gumbel, dtype=np.float32)
    tau_f = np.float32(np.asarray(tau))
    gamma = np.asarray(gamma, dtype=np.float32)
    beta = np.asarray(beta, dtype=np.float32)

    # replicate reference softmax/argmax in fp32 (argmax of z == argmax of
    # softmax(z); verified min top-2 gap 3.4e-4 for these inputs)
    z = (logits + gumbel) / tau_f                     # [2, CE, C1] fp32
    idx = z.argmax(axis=-1)                           # [2, CE]
    zm = z.max(axis=-1, keepdims=True)
    ez = np.exp(z - zm, dtype=np.float32)
    soft = ez / ez.sum(axis=-1, keepdims=True, dtype=np.float32)
    s_hot = np.take_along_axis(soft, idx[..., None], axis=-1)[..., 0]
    w = (np.float32(1.0) - s_hot) + s_hot             # [2, CE] (== 1.0 here)
    weff = (w[0] * w[1]).astype(np.float32)           # [CE]

    inv_n = np.float32(1.0) / np.float32(N)

    # channel-major copy of x for fast row gathers: [C1, B*L]
    xt = np.ascontiguousarray(
        x.reshape(B, C1, L).transpose(1, 0, 2)).reshape(C1, N)
    if GATHER_DTYPE == "f16":
        xt = xt.astype(np.float16)

    # R / R^T block one-hot (partition p belongs to e_sub = p//B)
    rmat = np.zeros((128, EG), dtype=np.float32)
    for es in range(EG):
        rmat[es * B:(es + 1) * B, es] = 1.0
    rtmat = np.ascontiguousarray(rmat.T)

    in_maps = []
    for k in range(NCORES):
        e0 = k * EPC
        rows = np.concatenate([idx[0, e0:e0 + EPC], idx[1, e0:e0 + EPC]])
        xsel = np.ascontiguousarray(xt[rows])         # [128, N]

        coef = np.zeros((EG, 4 * NG), dtype=np.float32)
        for g in range(NG):
            el = e0 + g * EG + np.arange(EG)          # global e for (g, e_sub)
            we = weff[el]
            coef[:, 0 * NG + g] = we * inv_n                  # w/N
            coef[:, 1 * NG + g] = we * we * inv_n             # w^2/N
            coef[:, 2 * NG + g] = gamma[el] * we              # gamma*w
            coef[:, 3 * NG + g] = beta[el]

        consts = np.zeros((128, CCOLS), dtype=np.float32)
        consts[:, 0:EG] = rmat
        consts[0:EG, 8:40] = coef
        consts[0:EG, 40:168] = rtmat
        consts[0:EG, 168] = BN_EPS

        in_maps.append({
            "xsel": xsel,
            "consts": consts,
        })
    return in_maps


def _install_ntff_shim():
    """The agent image's antenv lacks axon_hooks; recreate it so
    run_bass_kernel_spmd(trace=True) can capture NTFF profiles."""
    import types
    if "antenv.axon_hooks" in sys.modules:
        return
    mod = types.ModuleType("antenv.axon_hooks")
    _hook = [None]
    mod.set_axon_ntff_profile_hook = lambda h: _hook.__setitem__(0, h)
    mod.get_axon_ntff_profile_hook = lambda: _hook[0]
    sys.modules["antenv.axon_hooks"] = mod
    import antenv
    antenv.axon_hooks = mod
    from trn_agent_boot.trn_boot import _ntff_profile_via_ctypes
    mod.set_axon_ntff_profile_hook(
        _ntff_profile_via_ctypes("/opt/axon/libaxon_pjrt.so"))


def kernel(x, logits, gumbel, tau, gamma, beta):
    global LAST_RESULT
    nc = _get_program()
    in_maps = _host_prep(x, logits, gumbel, tau, gamma, beta)

    trace = bool(int(os.environ.get("KERNEL_PROFILE", "0")))
    if trace:
        try:
            _install_ntff_shim()
        except Exception:
            trace = False
    try:
        res = run_bass_kernel_spmd(nc, in_maps, list(range(NCORES)),
                                   trace=trace)
    except Exception:
        if not trace:
            raise
        res = run_bass_kernel_spmd(nc, in_maps, list(range(NCORES)),
                                   trace=False)
    LAST_RESULT = res

    out = np.empty((B, CE, L), dtype=np.float32)
    for k in range(NCORES):
        ok = res.results[k]["out"]
        if ok.dtype != np.float32:
            ok = ok.astype(np.float32)
        out[:, k * EPC:(k + 1) * EPC, :] = ok.transpose(1, 0, 2)
    return out.reshape(B, CE, H, W)
